# revision 44
# baseline (speedup 1.0000x reference)
"""NextVLAD Trainium2 kernel: 8-core SPMD bass/tile implementation (v3).

Strategy:
  * Host folds W_emb@W1 (and the centroid/attention projections) so the
    front end contracts over K=145 instead of K=2048.  All matmuls run
    in fp16 (full PE rate like bf16, but 8x less quantization noise;
    fp32 runs 2-4x slower on the PE).  Only the softmax exp output
    stays f32 (range).
  * Front end is data-parallel over batch (16 images/core, rows padded
    to 64 per image).  The VLAD einsum uses a block-diagonal act tile
    so two images share one 512-col PE stream; the act row-sum needed
    for the centroid term rides on a DVE-accumulated bd sum.
  * The trace is software-pipelined (h-phase of row-tile rt is emitted
    before the vlad-phase of rt-1) because engine queues execute in
    order: the PE must never sit behind the softmax dependency chain.
    DMA issue queues are load-balanced (sync: x/wh/scatters, scalar:
    second weight half + tail weights late, gpsimd: paced wcg stream)
    since a DMA issue that blocks mid-queue stalls everything behind it.
  * Each core writes its VLAD output pre-blocked by destination core;
    two AllToAll collectives (images 0-7 hidden under the front end,
    8-15 after) give every core the full batch for its own 4096-wide
    k-slice of the cg.fc1 contraction -- 1MB moved instead of an 8MB
    AllGather, and 32 PE transposes instead of 256.
  * Context gating: y_partial[128,2048] = vlads_slice^T @ wcg_slice,
    wcg (16.8MB fp16) fully prefetched into SBUF during the front end.
    One fp16 AllReduce combines the k-slice partials; the small tail
    (BN1, gating, fc2, logits) is computed replicated on every core in
    a transposed layout (stats via ones-vector matmuls), so no further
    collectives are needed.  BN invariances let bcg/bg1 be dropped and
    all BN scales folded on-chip.
"""

import numpy as np
import ml_dtypes

import concourse.bass as bass
import concourse.mybir as mybir
import concourse.tile as tile
from concourse import bacc, bass_utils

F32 = mybir.dt.float32
F32R = mybir.dt.float32r
BF16 = mybir.dt.bfloat16

B, T, POSE = 128, 60, 144
DIM, EXP, GRP, K, NCLS = 2048, 2, 8, 64, 10
ED = EXP * DIM            # 4096
FS = ED // GRP            # 512
VLAD = K * FS             # 32768
HID = DIM                 # 2048
RED = HID // 8            # 256

NCORES = 8
BPC = B // NCORES         # 16 images per core
TP = 64                   # padded rows per image (60 real + 4 pad)
ROWS = BPC * TP           # 1024 rows per core
RT = ROWS // 128          # 8 row tiles (2 images each)
KA = POSE + 1             # 145 contraction (with bias row)
SLC = VLAD // NCORES      # 4096 k-slice columns per core
KPC = K // NCORES         # 8 clusters per core slice
NKT = SLC // 128          # 32 k-tiles in the cg contraction
EPS = 1e-5

_CACHED = {}


def _build_nc(collectives=True):
    nc = bacc.Bacc("TRN2", target_bir_lowering=False, debug=False,
                   num_devices=NCORES)

    xT = nc.dram_tensor("xT", [KA, ROWS], BF16, kind="ExternalInput").ap()
    wh = nc.dram_tensor("wh", [KA, ED], BF16, kind="ExternalInput").ap()
    wl = nc.dram_tensor("wl", [KA, FS + GRP], BF16, kind="ExternalInput").ap()
    c2t2 = nc.dram_tensor("c2t2", [128, FS], F32, kind="ExternalInput").ap()
    wcg = nc.dram_tensor("wcg", [SLC, HID], BF16, kind="ExternalInput").ap()
    wg1 = nc.dram_tensor("wg1", [HID, RED], BF16, kind="ExternalInput").ap()
    wg2 = nc.dram_tensor("wg2", [RED, HID], BF16, kind="ExternalInput").ap()
    w3 = nc.dram_tensor("w3", [HID, NCLS], BF16, kind="ExternalInput").ap()
    bn1gT = nc.dram_tensor("bn1gT", [128, 16], F32, kind="ExternalInput").ap()
    bn1bT = nc.dram_tensor("bn1bT", [128, 16], F32, kind="ExternalInput").ap()
    bn2gT = nc.dram_tensor("bn2gT", [128, 2], F32, kind="ExternalInput").ap()
    bn2bT = nc.dram_tensor("bn2bT", [128, 2], F32, kind="ExternalInput").ap()
    bg2T = nc.dram_tensor("bg2T", [128, 16], F32, kind="ExternalInput").ap()
    rowmask = nc.dram_tensor("rowmask", [128, 1], F32, kind="ExternalInput").ap()
    identb_in = nc.dram_tensor("identb_in", [128, 128], BF16,
                               kind="ExternalInput").ap()
    identf_in = nc.dram_tensor("identf_in", [128, 128], F32,
                               kind="ExternalInput").ap()
    onesb_in = nc.dram_tensor("onesb_in", [128, 1], BF16,
                              kind="ExternalInput").ap()
    onesf_in = nc.dram_tensor("onesf_in", [128, 1], F32,
                              kind="ExternalInput").ap()
    outT = nc.dram_tensor("outT", [NCLS, B], F32, kind="ExternalOutput").ap()

    AF = mybir.ActivationFunctionType
    AX = mybir.AxisListType
    ALU = mybir.AluOpType
    RG = [list(range(NCORES))]

    with tile.TileContext(nc) as tc:
      with tc.tile_pool(name="const", bufs=1) as const, \
           tc.tile_pool(name="wstream", bufs=12) as wpool, \
           tc.tile_pool(name="dram", bufs=1, space="DRAM") as dram:
        # ---- constants loaded once ----
        wh0 = const.tile([128, ED], BF16)
        wh1 = const.tile([KA - 128, ED], BF16)
        nc.sync.dma_start(wh0[:], wh[0:128, :])
        nc.sync.dma_start(wh1[:], wh[128:KA, :])
        wl0 = const.tile([128, FS + GRP], BF16)
        wl1 = const.tile([KA - 128, FS + GRP], BF16)
        nc.scalar.dma_start(wl0[:], wl[0:128, :])
        nc.scalar.dma_start(wl1[:], wl[128:KA, :])
        c2t2_sb = const.tile([128, FS], F32)
        nc.scalar.dma_start(c2t2_sb[:], c2t2[:])
        rmask_sb = const.tile([128, 1], F32)
        nc.scalar.dma_start(rmask_sb[:], rowmask[:])
        identb = const.tile([128, 128], BF16)
        nc.scalar.dma_start(identb[:], identb_in[:])
        onesb = const.tile([128, 1], BF16)
        nc.scalar.dma_start(onesb[:], onesb_in[:])
        onesf = const.tile([128, 1], F32)
        nc.scalar.dma_start(onesf[:], onesf_in[:])
        eps1 = const.tile([128, 1], F32)
        nc.any.memset(eps1[:, :], EPS)

        # tail weights: wg1 as rhs tiles [128k, 256], wg2 as lhsT rows,
        # w3 as lhsT tiles [128k, 10]
        wg1_sb = const.tile([128, 16 * RED], BF16)
        nc.scalar.dma_start(wg1_sb[:].rearrange("p (kt n) -> p kt n", kt=16),
                            wg1[:].rearrange("(kt p) n -> p kt n", p=128))
        wg2_sb = []
        for kt in range(2):
            t = const.tile([128, HID], BF16, tag=f"wg2_{kt}")
            nc.scalar.dma_start(t[:], wg2[kt * 128:(kt + 1) * 128, :])
            wg2_sb.append(t)
        w3_sb = const.tile([128, 16 * NCLS], BF16)
        nc.scalar.dma_start(w3_sb[:].rearrange("p (kt n) -> p kt n", kt=16),
                            w3[:].rearrange("(kt p) n -> p kt n", p=128))

        # block-diagonal act tiles (off-diagonal stays zero forever)
        bd_tiles = []
        for i in range(3):
            t = const.tile([128, 128], BF16, tag=f"bd{i}")
            nc.any.memset(t[:, :], 0.0)
            bd_tiles.append(t)

        # dram scratch
        vlA = dram.tile([64, SLC], BF16)
        vlB = dram.tile([64, SLC], BF16)
        shared = "Shared" if collectives else "Local"
        slcA = dram.tile([64, SLC], BF16)
        slcB = dram.tile([64, SLC], BF16)
        ypart = dram.tile([B, HID], F16)
        y_all = dram.tile([B, HID], F16, addr_space=shared)

        # ================= front end =================
        with tc.tile_pool(name="fex", bufs=3) as xpool, \
             tc.tile_pool(name="feh", bufs=2) as hpool, \
             tc.tile_pool(name="feact", bufs=2) as apool, \
             tc.tile_pool(name="feaux", bufs=3) as aux, \
             tc.tile_pool(name="fevo", bufs=2) as vopool, \
             tc.tile_pool(name="ph", bufs=3, space="PSUM") as phpool, \
             tc.tile_pool(name="pl", bufs=2, space="PSUM") as plpool, \
             tc.tile_pool(name="pv", bufs=2, space="PSUM") as pvpool:
            wch_tiles = []
            _pace = [0, 5, 5, 5, 5, 4, 4, 4]
            # preload every row-tile's x up front (tiny) so no later
            # sync-queue stall (scatters block on compute) can starve the
            # h matmuls of weights.
            xk_tiles = {}
            for rt in range(RT):
                rs = rt * 128
                xk0 = xpool.tile([128, 128], F16, tag=f"xk0_{rt}", bufs=1)
                xk1 = xpool.tile([KA - 128, 128], F16, tag=f"xk1_{rt}",
                                 bufs=1)
                eng = nc.sync if rt < 4 else nc.scalar
                eng.dma_start(xk0[:], xT[0:128, rs:rs + 128])
                eng.dma_start(xk1[:], xT[128:KA, rs:rs + 128])
                xk_tiles[rt] = (xk0, xk1)
            state = {}

            def h_phase(rt):
                # paced wcg prefetch (gpsimd queue, separate hw queue from
                # the sync-queue scatters)
                for _q in range(_pace[rt]):
                    kt = len(wch_tiles)
                    wch = wpool.tile([128, HID], F16, tag=f"wch{kt}",
                                     name=f"wch{kt}", bufs=1)
                    nc.gpsimd.dma_start(wch[:],
                                        wcg[kt * 128:(kt + 1) * 128, :])
                    wch_tiles.append(wch)
                xk0, xk1 = xk_tiles.pop(rt)
                h_sb = hpool.tile([128, ED], F16, tag="h", bufs=3)
                for nt in range(ED // 1024):
                    # 1024-wide chunk spans two PSUM banks; each matmul dst
                    # stays inside one bank.  Halves the copy count and the
                    # PE<->copy semaphore round-trips that pace this loop.
                    ph = phpool.tile([128, 1024], F32, tag="ph", bufs=2)
                    for half in range(2):
                        cs = nt * 1024 + half * 512
                        wh0t = wh0a if cs < ED // 2 else wh0b
                        co = cs % (ED // 2)
                        nc.tensor.matmul(ph[:, half * 512:(half + 1) * 512],
                                         xk0[:], wh0t[:, co:co + 512],
                                         start=True, stop=False,
                                         skip_group_check=(half > 0))
                        nc.tensor.matmul(ph[:, half * 512:(half + 1) * 512],
                                         xk1[:], wh1[:, cs:cs + 512],
                                         start=False, stop=True,
                                         skip_group_check=(half > 0))
                    if nt in (1, 3):
                        nc.vector.tensor_copy(
                            h_sb[:, nt * 1024:(nt + 1) * 1024], ph[:])
                    else:
                        nc.scalar.copy(h_sb[:, nt * 1024:(nt + 1) * 1024],
                                       ph[:])
                pl = plpool.tile([128, 512], F32, tag="pl", bufs=1)
                nc.tensor.matmul(pl[:], xk0[:], wl0[:, 0:512],
                                 start=True, stop=False)
                nc.tensor.matmul(pl[:], xk1[:], wl1[:, 0:512],
                                 start=False, stop=True)
                pa = plpool.tile([128, GRP], F32, tag="pa", bufs=1)
                nc.tensor.matmul(pa[:], xk0[:], wl0[:, 512:512 + GRP],
                                 start=True, stop=False)
                nc.tensor.matmul(pa[:], xk1[:], wl1[:, 512:512 + GRP],
                                 start=False, stop=True)

                # softmax without max-subtraction: logits are O(30) so
                # exp() stays well inside f32 range.
                act_raw = apool.tile([128, 512], F32, tag="act", bufs=2)
                nc.scalar.activation(act_raw[:], pl[:], AF.Exp)
                att = aux.tile([128, GRP], F32, tag="att")
                nc.scalar.activation(att[:], pa[:], AF.Exp, scale=-1.0)
                nc.vector.tensor_scalar_add(att[:], att[:], 1.0)
                sums = aux.tile([128, GRP], F32, tag="sums")
                nc.vector.reduce_sum(
                    sums[:].rearrange("p (g o) -> p g o", o=1),
                    act_raw[:].rearrange("p (g k) -> p g k", g=GRP),
                    axis=AX.X)
                den = aux.tile([128, GRP], F32, tag="den")
                nc.vector.tensor_mul(den[:], att[:], sums[:])
                scl = aux.tile([128, GRP], F32, tag="scl")
                nc.vector.reciprocal(scl[:], den[:])
                nc.vector.tensor_scalar_mul(scl[:], scl[:], rmask_sb[:])
                state[rt] = (h_sb, act_raw, scl)

            def vlad_phase(rt):
                h_sb, act_raw, scl = state.pop(rt)
                bg = rt * 2
                pv = pvpool.tile([128, FS], F32, tag="pv", bufs=1)
                s128 = pvpool.tile([128, 1], F32, tag="s128", bufs=1)
                for g in range(GRP):
                    bd = bd_tiles[g % 3]
                    nc.vector.tensor_scalar_mul(
                        bd[0:64, 0:64],
                        act_raw[0:64, g * 64:(g + 1) * 64],
                        scl[0:64, g:g + 1])
                    nc.vector.tensor_scalar_mul(
                        bd[64:128, 64:128],
                        act_raw[64:128, g * 64:(g + 1) * 64],
                        scl[64:128, g:g + 1])
                    nc.tensor.matmul(pv[:], bd[:],
                                     h_sb[:, g * FS:(g + 1) * FS],
                                     start=(g == 0), stop=(g == GRP - 1))
                    if g == 0:
                        nc.vector.tensor_copy(bdsum[:], bd[:])
                    else:
                        nc.vector.tensor_add(bdsum[:], bdsum[:], bd[:])
                nc.tensor.matmul(s128[:], bdsum[:], onesb[:],
                                 start=True, stop=True,
                                 skip_group_check=True)
                s_sb = aux.tile([128, 1], F32, tag="s_sb")
                nc.vector.tensor_copy(s_sb[:], s128[:])
                tmp = vopool.tile([128, FS], F32, tag="tmpc2")
                nc.vector.tensor_scalar_mul(tmp[:], c2t2_sb[:], s_sb[:, 0:1])
                vout = vopool.tile([128, FS], F16, tag="vout")
                nc.vector.tensor_sub(vout[:], pv[:], tmp[:])

                # scatter k-slices to the a2a input, pre-blocked by
                # destination core (sync queue; xk all pre-issued).
                vdst = vlA if rt < 4 else vlB
                ib = bg if rt < 4 else bg - 8
                for c in range(NCORES):
                    for b2 in range(2):
                        r = c * 8 + ib + b2
                        nc.sync.dma_start(
                            vdst[r:r + 1, :].rearrange(
                                "o (kk f) -> (o kk) f", kk=KPC),
                            vout[b2 * 64 + c * KPC:
                                 b2 * 64 + (c + 1) * KPC, :])

            # software-pipelined trace: h(rt+1) is emitted before
            # vlad(rt) so the in-order PE queue never waits on the
            # softmax chain.
            for rt in range(RT):
                h_phase(rt)
                if rt >= 1:
                    vlad_phase(rt - 1)
                if rt == 4 and collectives:
                    # vlA complete after vlad_phase(3); trigger hidden
                    nc.gpsimd.collective_compute(
                        "AllToAll", ALU.bypass, replica_groups=RG,
                        ins=[vlA.opt()], outs=[slcA.opt()])
            vlad_phase(RT - 1)
            if not collectives:
                nc.sync.dma_start(slcA[:, :], vlA[:, :])

            if collectives:
                nc.gpsimd.collective_compute(
                    "AllToAll", ALU.bypass, replica_groups=RG,
                    ins=[vlB.opt()], outs=[slcB.opt()])
            else:
                nc.sync.dma_start(slcB[:, :], vlB[:, :])

        # ================= context gating =================
        with tc.tile_pool(name="cgv", bufs=3) as vpool, \
             tc.tile_pool(name="cgsb", bufs=2) as cgsb, \
             tc.tile_pool(name="cgp", bufs=1, space="PSUM") as cgps, \
             tc.tile_pool(name="cgpt", bufs=2, space="PSUM") as cgpt:
            py = [cgps.tile([128, 512], F32, tag=f"py{i}", name=f"py{i}",
                            bufs=1) for i in range(4)]
            for kt in range(NKT):
                wch = wpool.tile([128, HID], BF16, tag="wch", name="wch",
                                 bufs=12)
                nc.sync.dma_start(wch[:], wcg[kt * 128:(kt + 1) * 128, :])
                vload = vpool.tile([128, 128], BF16, tag="vload",
                                   name="vload", bufs=3)
                nc.sync.dma_start(vload[0:64, :],
                                  slcA[:, kt * 128:(kt + 1) * 128])
                nc.sync.dma_start(vload[64:128, :],
                                  slcB[:, kt * 128:(kt + 1) * 128])
                pt = cgpt.tile([128, 128], BF16, tag="pt", bufs=2)
                nc.tensor.transpose(pt[:], vload[:], identb[:])
                vt = vpool.tile([128, 128], BF16, tag="vt", name="vt",
                                bufs=3)
                nc.vector.tensor_copy(vt[:], pt[:])
                for ch in range(4):
                    nc.tensor.matmul(py[ch][:], vt[:],
                                     wch[:, ch * 512:(ch + 1) * 512],
                                     start=(kt == 0), stop=(kt == NKT - 1),
                                     skip_group_check=(ch > 0))
            for ch in range(4):
                ych = cgsb.tile([128, 512], F16, tag="ych")
                nc.vector.tensor_copy(ych[:], py[ch][:])
                nc.sync.dma_start(ypart[:, ch * 512:(ch + 1) * 512], ych[:])

        if collectives:
            nc.gpsimd.collective_compute(
                "AllReduce", ALU.add, replica_groups=RG,
                ins=[ypart.opt()], outs=[y_all.opt()])
        else:
            nc.sync.dma_start(y_all[:, :], ypart[:, :])

        # ================= replicated tail =================
        with tc.tile_pool(name="tsb", bufs=1) as tsb, \
             tc.tile_pool(name="taux", bufs=2) as taux, \
             tc.tile_pool(name="tps", bufs=1, space="PSUM") as tps, \
             tc.tile_pool(name="tpt", bufs=2, space="PSUM") as tpt:
            y_sb = tsb.tile([128, HID], F16, tag="y_sb")
            nc.sync.dma_start(y_sb[:], y_all[:])
            sq = tsb.tile([128, HID], F16, tag="sq")
            for ch in range(4):
                nc.vector.tensor_mul(sq[:, ch * 512:(ch + 1) * 512],
                                     y_sb[:, ch * 512:(ch + 1) * 512],
                                     y_sb[:, ch * 512:(ch + 1) * 512])

            # per-column stats via N=1 matmuls with a ones vector:
            # pstat[:, ct] = sum_b y[b, ct*128+p], [:, 16+ct] = sum y^2
            pstat = tps.tile([128, 32], F32, tag="pstat", bufs=1)
            for ct in range(16):
                nc.tensor.matmul(pstat[:, ct:ct + 1],
                                 y_sb[:, ct * 128:(ct + 1) * 128], onesb[:],
                                 start=True, stop=True,
                                 skip_group_check=(ct > 0))
            for ct in range(16):
                nc.tensor.matmul(pstat[:, 16 + ct:17 + ct],
                                 sq[:, ct * 128:(ct + 1) * 128], onesb[:],
                                 start=True, stop=True,
                                 skip_group_check=True)
            mu = taux.tile([128, 16], F32, tag="mu")
            nc.vector.tensor_scalar_mul(mu[:], pstat[:, 0:16], 1.0 / B)
            ex2 = taux.tile([128, 16], F32, tag="ex2")
            nc.vector.tensor_scalar_mul(ex2[:], pstat[:, 16:32], 1.0 / B)
            musq = taux.tile([128, 16], F32, tag="musq")
            nc.vector.tensor_mul(musq[:], mu[:], mu[:])
            var = taux.tile([128, 16], F32, tag="var")
            nc.vector.tensor_sub(var[:], ex2[:], musq[:])
            sd = taux.tile([128, 16], F32, tag="sd")
            nc.scalar.activation(sd[:], var[:], AF.Sqrt, bias=eps1[:, 0:1])
            rstd = taux.tile([128, 16], F32, tag="rstd")
            nc.vector.reciprocal(rstd[:], sd[:])
            seff = taux.tile([128, 16], F32, tag="seff")
            nc.vector.tensor_mul(seff[:], bn1gT_sb[:], rstd[:])
            mue = taux.tile([128, 16], F32, tag="mue")
            nc.vector.tensor_mul(mue[:], mu[:], seff[:])
            beff = taux.tile([128, 16], F32, tag="beff")
            nc.vector.tensor_sub(beff[:], bn1bT_sb[:], mue[:])

            # transpose y tile-by-tile and normalize: ybnT[ct] is
            # [128 cols, 128 imgs] bf16
            ybnT = []
            for ct in range(16):
                ptr = tpt.tile([128, 128], F16, tag="ptr", bufs=2)
                nc.tensor.transpose(ptr[:], y_sb[:, ct * 128:(ct + 1) * 128],
                                    identb[:])
                yt = tsb.tile([128, 128], BF16, tag=f"ybnT{ct}",
                              name=f"ybnT{ct}")
                nc.vector.tensor_scalar(yt[:], ptr[:], seff[:, ct:ct + 1],
                                        beff[:, ct:ct + 1], ALU.mult,
                                        ALU.add)
                ybnT.append(yt)

            # z = ybn @ Wg1  [128 imgs, 256]
            pz = tps.tile([128, RED], F32, tag="pz", bufs=1)
            for ct in range(16):
                nc.tensor.matmul(pz[:], ybnT[ct][:],
                                 wg1_sb[:, ct * RED:(ct + 1) * RED],
                                 start=(ct == 0), stop=(ct == 15))
            z_sb = tsb.tile([128, RED], F16, tag="z_sb")
            nc.vector.tensor_copy(z_sb[:], pz[:])
            sq2 = tsb.tile([128, RED], F16, tag="sq2")
            nc.vector.tensor_mul(sq2[:], z_sb[:], z_sb[:])
            pstat2 = tps.tile([128, 4], F32, tag="pstat2", bufs=1)
            for j in range(2):
                nc.tensor.matmul(pstat2[:, j:j + 1],
                                 z_sb[:, j * 128:(j + 1) * 128], onesb[:],
                                 start=True, stop=True,
                                 skip_group_check=(j > 0))
                nc.tensor.matmul(pstat2[:, 2 + j:3 + j],
                                 sq2[:, j * 128:(j + 1) * 128], onesb[:],
                                 start=True, stop=True,
                                 skip_group_check=True)
            mu2 = taux.tile([128, 2], F32, tag="mu2")
            nc.vector.tensor_scalar_mul(mu2[:], pstat2[:, 0:2], 1.0 / B)
            ex22 = taux.tile([128, 2], F32, tag="ex22")
            nc.vector.tensor_scalar_mul(ex22[:], pstat2[:, 2:4], 1.0 / B)
            musq2 = taux.tile([128, 2], F32, tag="musq2")
            nc.vector.tensor_mul(musq2[:], mu2[:], mu2[:])
            var2 = taux.tile([128, 2], F32, tag="var2")
            nc.vector.tensor_sub(var2[:], ex22[:], musq2[:])
            sd2 = taux.tile([128, 2], F32, tag="sd2")
            nc.scalar.activation(sd2[:], var2[:], AF.Sqrt, bias=eps1[:, 0:1])
            rstd2 = taux.tile([128, 2], F32, tag="rstd2")
            nc.vector.reciprocal(rstd2[:], sd2[:])
            seff2 = taux.tile([128, 2], F32, tag="seff2")
            nc.vector.tensor_mul(seff2[:], bn2gT_sb[:], rstd2[:])
            mue2 = taux.tile([128, 2], F32, tag="mue2")
            nc.vector.tensor_mul(mue2[:], mu2[:], seff2[:])
            beff2 = taux.tile([128, 2], F32, tag="beff2")
            nc.vector.tensor_sub(beff2[:], bn2bT_sb[:], mue2[:])

            rT = []
            for j in range(2):
                ptr = tpt.tile([128, 128], F16, tag="ptr", bufs=2)
                nc.tensor.transpose(ptr[:], z_sb[:, j * 128:(j + 1) * 128],
                                    identb[:])
                tz = taux.tile([128, 128], F32, tag="tz")
                nc.vector.tensor_scalar(tz[:], ptr[:], seff2[:, j:j + 1],
                                        beff2[:, j:j + 1], ALU.mult,
                                        ALU.add)
                rt_ = tsb.tile([128, 128], BF16, tag=f"rT{j}", name=f"rT{j}")
                nc.vector.tensor_scalar_max(rt_[:], tz[:], 0.0)
                rT.append(rt_)

            # gateT = sigmoid(Wg2^T @ r^T + bg2), oT = ybnT * gateT,
            # outT = W3^T @ oT  -- all in the transposed layout.
            po = tps.tile([NCLS, B], F32, tag="po", bufs=1)
            for m in range(16):
                pg = tpt.tile([128, 128], F32, tag="pg", bufs=2)
                for j in range(2):
                    nc.tensor.matmul(pg[:], wg2_sb[j][:, m * 128:(m + 1) * 128],
                                     rT[j][:], start=(j == 0), stop=(j == 1))
                gt = taux.tile([128, 128], BF16, tag="gt")
                nc.scalar.activation(gt[:], pg[:], AF.Sigmoid,
                                     bias=bg2T_sb[:, m:m + 1])
                ot = taux.tile([128, 128], BF16, tag="ot")
                nc.vector.tensor_mul(ot[:], ybnT[m][:], gt[:])
                nc.tensor.matmul(po[:], w3_sb[:, m * NCLS:(m + 1) * NCLS],
                                 ot[:], start=(m == 0), stop=(m == 15),
                                 skip_group_check=True)
            out_sb = taux.tile([NCLS, B], F32, tag="outp")
            nc.vector.tensor_copy(out_sb[:], po[:])
            nc.sync.dma_start(outT[:], out_sb[:])

    nc.compile()
    return nc


def _host_prep(inputs):
    f32 = np.float32
    bf16 = np.float16
    g = {k: np.asarray(v, dtype=f32) for k, v in inputs.items()}

    x2 = np.transpose(g["x"], (0, 3, 1, 2)).reshape(B, T, POSE)
    Wh = g["W_emb"] @ g["W1"]                       # [144, 4096]
    bh = g["b_emb"] @ g["W1"] + g["b1"]             # [4096]
    C1cat = np.concatenate([g["centroids1"], g["W2"]], axis=1)  # [4096, 520]
    WL = Wh @ C1cat                                 # [144, 520]
    bL = bh @ C1cat
    bL[FS:] += g["b2"]
    wh_aug = np.concatenate([Wh, bh[None, :]], axis=0).astype(bf16)
    wl_aug = np.concatenate([WL, bL[None, :]], axis=0).astype(bf16)
    c2t = np.ascontiguousarray(g["centroids2"][0].T)            # [64, 512]
    c2t2 = np.concatenate([c2t, c2t], axis=0)                   # [128, 512]

    # permute Wcg rows: our flat vlad index is k*FS+f, reference is f*K+k
    new = np.arange(VLAD)
    old = (new % FS) * K + (new // FS)
    wcg_perm = g["Wcg"][old, :].astype(bf16)        # [32768, 2048]

    rmask = np.zeros((128, 1), f32)
    rmask[:T] = 1.0
    rmask[TP:TP + T] = 1.0
    identb_np = np.eye(128, dtype=np.float16)
    identf_np = np.eye(128, dtype=f32)
    onesb_np = np.ones((128, 1), np.float16)
    onesf_np = np.ones((128, 1), f32)
    common = {
        "wh": wh_aug,
        "wl": wl_aug,
        "c2t2": c2t2,
        "wg1": g["Wg1"].astype(bf16),
        "wg2": g["Wg2"].astype(bf16),
        "w3": g["W3"].astype(bf16),
        "bn1gT": np.ascontiguousarray(g["g_bn1"].reshape(16, 128).T),
        "bn1bT": np.ascontiguousarray(g["b_bn1"].reshape(16, 128).T),
        "bn2gT": np.ascontiguousarray(g["g_bn2"].reshape(2, 128).T),
        "bn2bT": np.ascontiguousarray(g["b_bn2"].reshape(2, 128).T),
        "bg2T": np.ascontiguousarray(g["bg2"].reshape(16, 128).T),
        "rowmask": rmask,
        "identb_in": identb_np,
        "identf_in": identf_np,
        "onesb_in": onesb_np,
        "onesf_in": onesf_np,
    }
    in_maps = []
    for c in range(NCORES):
        xs = x2[c * BPC:(c + 1) * BPC]              # [16, 60, 144]
        xp = np.zeros((BPC, TP, POSE), f32)
        xp[:, :T] = xs
        xTf = np.ones((KA, ROWS), f32)
        xTf[:POSE] = xp.reshape(ROWS, POSE).T
        m = dict(common)
        m["xT"] = xTf.astype(bf16)
        m["wcg"] = np.ascontiguousarray(wcg_perm[c * SLC:(c + 1) * SLC, :])
        in_maps.append(m)
    return in_maps, g["b3"]


# vstage row r <-> global batch index (r<64: core r//8 img r%8,
# r>=64: core (r-64)//8 img 8+(r-64)%8)
_PERM = np.zeros(B, np.int64)
for _r in range(B):
    if _r < 64:
        _s, _i = divmod(_r, 8)
        _PERM[_r] = BPC * _s + _i
    else:
        _s, _i = divmod(_r - 64, 8)
        _PERM[_r] = BPC * _s + 8 + _i


def kernel(**inputs):
    if "nc" not in _CACHED:
        _CACHED["nc"] = _build_nc()
    nc = _CACHED["nc"]
    in_maps, b3 = _host_prep(inputs)
    res = bass_utils.run_bass_kernel_spmd(nc, in_maps,
                                          core_ids=list(range(NCORES)))
    _CACHED["last_res"] = res
    out = np.zeros((B, NCLS), np.float32)
    out[_PERM, :] = res.results[0]["outT"].T
    return out + b3[None, :]


# revision 45
# speedup vs baseline: 1.0167x; 1.0167x over previous
"""NextVLAD Trainium2 kernel: 8-core SPMD bass/tile implementation (v3).

Strategy:
  * Host folds W_emb@W1 (and the centroid/attention projections) so the
    front end contracts over K=145 instead of K=2048.  All matmuls run
    in fp16 (full PE rate like bf16, but 8x less quantization noise;
    fp32 runs 2-4x slower on the PE).  Only the softmax exp output
    stays f32 (range).
  * Front end is data-parallel over batch (16 images/core, rows padded
    to 64 per image).  The VLAD einsum uses a block-diagonal act tile
    so two images share one 512-col PE stream; the act row-sum needed
    for the centroid term rides on a DVE-accumulated bd sum.
  * The trace is software-pipelined (h-phase of row-tile rt is emitted
    before the vlad-phase of rt-1) because engine queues execute in
    order: the PE must never sit behind the softmax dependency chain.
    DMA issue queues are load-balanced (sync: x/wh/scatters, scalar:
    second weight half + tail weights late, gpsimd: paced wcg stream)
    since a DMA issue that blocks mid-queue stalls everything behind it.
  * Each core writes its VLAD output pre-blocked by destination core;
    two AllToAll collectives (images 0-7 hidden under the front end,
    8-15 after) give every core the full batch for its own 4096-wide
    k-slice of the cg.fc1 contraction -- 1MB moved instead of an 8MB
    AllGather, and 32 PE transposes instead of 256.
  * Context gating: y_partial[128,2048] = vlads_slice^T @ wcg_slice,
    wcg (16.8MB fp16) fully prefetched into SBUF during the front end.
    One fp16 AllReduce combines the k-slice partials; the small tail
    (BN1, gating, fc2, logits) is computed replicated on every core in
    a transposed layout (stats via ones-vector matmuls), so no further
    collectives are needed.  BN invariances let bcg/bg1 be dropped and
    all BN scales folded on-chip.
"""

import numpy as np
import ml_dtypes

import concourse.bass as bass
import concourse.mybir as mybir
import concourse.tile as tile
from concourse import bacc, bass_utils

F32 = mybir.dt.float32
F32R = mybir.dt.float32r
BF16 = mybir.dt.bfloat16

B, T, POSE = 128, 60, 144
DIM, EXP, GRP, K, NCLS = 2048, 2, 8, 64, 10
ED = EXP * DIM            # 4096
FS = ED // GRP            # 512
VLAD = K * FS             # 32768
HID = DIM                 # 2048
RED = HID // 8            # 256

NCORES = 8
BPC = B // NCORES         # 16 images per core
TP = 64                   # padded rows per image (60 real + 4 pad)
ROWS = BPC * TP           # 1024 rows per core
RT = ROWS // 128          # 8 row tiles (2 images each)
KA = POSE + 1             # 145 contraction (with bias row)
SLC = VLAD // NCORES      # 4096 k-slice columns per core
KPC = K // NCORES         # 8 clusters per core slice
NKT = SLC // 128          # 32 k-tiles in the cg contraction
EPS = 1e-5

_CACHED = {}


def _build_nc(collectives=True):
    nc = bacc.Bacc("TRN2", target_bir_lowering=False, debug=False,
                   num_devices=NCORES)

    xT = nc.dram_tensor("xT", [KA, ROWS], BF16, kind="ExternalInput").ap()
    wh = nc.dram_tensor("wh", [KA, ED], BF16, kind="ExternalInput").ap()
    wl = nc.dram_tensor("wl", [KA, FS + GRP], BF16, kind="ExternalInput").ap()
    c2t2 = nc.dram_tensor("c2t2", [128, FS], F32, kind="ExternalInput").ap()
    wcg = nc.dram_tensor("wcg", [SLC, HID], BF16, kind="ExternalInput").ap()
    wg1 = nc.dram_tensor("wg1", [HID, RED], BF16, kind="ExternalInput").ap()
    wg2 = nc.dram_tensor("wg2", [RED, HID], BF16, kind="ExternalInput").ap()
    w3 = nc.dram_tensor("w3", [HID, NCLS], BF16, kind="ExternalInput").ap()
    bn1gT = nc.dram_tensor("bn1gT", [128, 16], F32, kind="ExternalInput").ap()
    bn1bT = nc.dram_tensor("bn1bT", [128, 16], F32, kind="ExternalInput").ap()
    bn2gT = nc.dram_tensor("bn2gT", [128, 2], F32, kind="ExternalInput").ap()
    bn2bT = nc.dram_tensor("bn2bT", [128, 2], F32, kind="ExternalInput").ap()
    bg2T = nc.dram_tensor("bg2T", [128, 16], F32, kind="ExternalInput").ap()
    rowmask = nc.dram_tensor("rowmask", [128, 1], F32, kind="ExternalInput").ap()
    identb_in = nc.dram_tensor("identb_in", [128, 128], BF16,
                               kind="ExternalInput").ap()
    identf_in = nc.dram_tensor("identf_in", [128, 128], F32,
                               kind="ExternalInput").ap()
    onesb_in = nc.dram_tensor("onesb_in", [128, 1], BF16,
                              kind="ExternalInput").ap()
    onesf_in = nc.dram_tensor("onesf_in", [128, 1], F32,
                              kind="ExternalInput").ap()
    outT = nc.dram_tensor("outT", [NCLS, B], F32, kind="ExternalOutput").ap()

    AF = mybir.ActivationFunctionType
    AX = mybir.AxisListType
    ALU = mybir.AluOpType
    RG = [list(range(NCORES))]

    with tile.TileContext(nc) as tc:
      with tc.tile_pool(name="const", bufs=1) as const, \
           tc.tile_pool(name="wstream", bufs=12) as wpool, \
           tc.tile_pool(name="dram", bufs=1, space="DRAM") as dram:
        # ---- constants loaded once ----
        wh0 = const.tile([128, ED], BF16)
        wh1 = const.tile([KA - 128, ED], BF16)
        nc.sync.dma_start(wh0[:], wh[0:128, :])
        nc.sync.dma_start(wh1[:], wh[128:KA, :])
        wl0 = const.tile([128, FS + GRP], BF16)
        wl1 = const.tile([KA - 128, FS + GRP], BF16)
        nc.scalar.dma_start(wl0[:], wl[0:128, :])
        nc.scalar.dma_start(wl1[:], wl[128:KA, :])
        c2t2_sb = const.tile([128, FS], F32)
        nc.scalar.dma_start(c2t2_sb[:], c2t2[:])
        rmask_sb = const.tile([128, 1], F32)
        nc.scalar.dma_start(rmask_sb[:], rowmask[:])
        identb = const.tile([128, 128], BF16)
        nc.scalar.dma_start(identb[:], identb_in[:])
        onesb = const.tile([128, 1], BF16)
        nc.scalar.dma_start(onesb[:], onesb_in[:])
        onesf = const.tile([128, 1], F32)
        nc.scalar.dma_start(onesf[:], onesf_in[:])
        eps1 = const.tile([128, 1], F32)
        nc.any.memset(eps1[:, :], EPS)

        # tail weights: wg1 as rhs tiles [128k, 256], wg2 as lhsT rows,
        # w3 as lhsT tiles [128k, 10]
        wg1_sb = const.tile([128, 16 * RED], BF16)
        nc.scalar.dma_start(wg1_sb[:].rearrange("p (kt n) -> p kt n", kt=16),
                            wg1[:].rearrange("(kt p) n -> p kt n", p=128))
        wg2_sb = []
        for kt in range(2):
            t = const.tile([128, HID], BF16, tag=f"wg2_{kt}")
            nc.scalar.dma_start(t[:], wg2[kt * 128:(kt + 1) * 128, :])
            wg2_sb.append(t)
        w3_sb = const.tile([128, 16 * NCLS], BF16)
        nc.scalar.dma_start(w3_sb[:].rearrange("p (kt n) -> p kt n", kt=16),
                            w3[:].rearrange("(kt p) n -> p kt n", p=128))

        # block-diagonal act tiles (off-diagonal stays zero forever)
        bd_tiles = []
        for i in range(3):
            t = const.tile([128, 128], BF16, tag=f"bd{i}")
            nc.any.memset(t[:, :], 0.0)
            bd_tiles.append(t)

        # dram scratch
        vlA = dram.tile([64, SLC], BF16)
        vlB = dram.tile([64, SLC], BF16)
        shared = "Shared" if collectives else "Local"
        slcA = dram.tile([64, SLC], BF16)
        slcB = dram.tile([64, SLC], BF16)
        ypart = dram.tile([B, HID], F16)
        y_all = dram.tile([B, HID], F16, addr_space=shared)

        # ================= front end =================
        with tc.tile_pool(name="fex", bufs=3) as xpool, \
             tc.tile_pool(name="feh", bufs=2) as hpool, \
             tc.tile_pool(name="feact", bufs=2) as apool, \
             tc.tile_pool(name="feaux", bufs=3) as aux, \
             tc.tile_pool(name="fevo", bufs=2) as vopool, \
             tc.tile_pool(name="ph", bufs=3, space="PSUM") as phpool, \
             tc.tile_pool(name="pl", bufs=2, space="PSUM") as plpool, \
             tc.tile_pool(name="pv", bufs=2, space="PSUM") as pvpool:
            wch_tiles = []
            _pace = [0, 6, 6, 6, 0, 5, 5, 4]
            # preload every row-tile's x up front (tiny) so no later
            # sync-queue stall (scatters block on compute) can starve the
            # h matmuls of weights.
            xk_tiles = {}
            for rt in range(RT):
                rs = rt * 128
                xk0 = xpool.tile([128, 128], F16, tag=f"xk0_{rt}", bufs=1)
                xk1 = xpool.tile([KA - 128, 128], F16, tag=f"xk1_{rt}",
                                 bufs=1)
                eng = nc.sync if rt < 4 else nc.scalar
                eng.dma_start(xk0[:], xT[0:128, rs:rs + 128])
                eng.dma_start(xk1[:], xT[128:KA, rs:rs + 128])
                xk_tiles[rt] = (xk0, xk1)
            state = {}

            def h_phase(rt):
                # paced wcg prefetch (gpsimd queue, separate hw queue from
                # the sync-queue scatters)
                for _q in range(_pace[rt]):
                    kt = len(wch_tiles)
                    wch = wpool.tile([128, HID], F16, tag=f"wch{kt}",
                                     name=f"wch{kt}", bufs=1)
                    nc.gpsimd.dma_start(wch[:],
                                        wcg[kt * 128:(kt + 1) * 128, :])
                    wch_tiles.append(wch)
                xk0, xk1 = xk_tiles.pop(rt)
                h_sb = hpool.tile([128, ED], F16, tag="h", bufs=3)
                for nt in range(ED // 1024):
                    # 1024-wide chunk spans two PSUM banks; each matmul dst
                    # stays inside one bank.  Halves the copy count and the
                    # PE<->copy semaphore round-trips that pace this loop.
                    ph = phpool.tile([128, 1024], F32, tag="ph", bufs=2)
                    for half in range(2):
                        cs = nt * 1024 + half * 512
                        wh0t = wh0a if cs < ED // 2 else wh0b
                        co = cs % (ED // 2)
                        nc.tensor.matmul(ph[:, half * 512:(half + 1) * 512],
                                         xk0[:], wh0t[:, co:co + 512],
                                         start=True, stop=False,
                                         skip_group_check=(half > 0))
                        nc.tensor.matmul(ph[:, half * 512:(half + 1) * 512],
                                         xk1[:], wh1[:, cs:cs + 512],
                                         start=False, stop=True,
                                         skip_group_check=(half > 0))
                    if nt in (1, 3):
                        nc.vector.tensor_copy(
                            h_sb[:, nt * 1024:(nt + 1) * 1024], ph[:])
                    else:
                        nc.scalar.copy(h_sb[:, nt * 1024:(nt + 1) * 1024],
                                       ph[:])
                pl = plpool.tile([128, 512], F32, tag="pl", bufs=1)
                nc.tensor.matmul(pl[:], xk0[:], wl0[:, 0:512],
                                 start=True, stop=False)
                nc.tensor.matmul(pl[:], xk1[:], wl1[:, 0:512],
                                 start=False, stop=True)
                pa = plpool.tile([128, GRP], F32, tag="pa", bufs=1)
                nc.tensor.matmul(pa[:], xk0[:], wl0[:, 512:512 + GRP],
                                 start=True, stop=False)
                nc.tensor.matmul(pa[:], xk1[:], wl1[:, 512:512 + GRP],
                                 start=False, stop=True)

                # softmax without max-subtraction: logits are O(30) so
                # exp() stays well inside f32 range.
                act_raw = apool.tile([128, 512], F32, tag="act", bufs=2)
                nc.scalar.activation(act_raw[:], pl[:], AF.Exp)
                att = aux.tile([128, GRP], F32, tag="att")
                nc.scalar.activation(att[:], pa[:], AF.Exp, scale=-1.0)
                nc.vector.tensor_scalar_add(att[:], att[:], 1.0)
                sums = aux.tile([128, GRP], F32, tag="sums")
                nc.vector.reduce_sum(
                    sums[:].rearrange("p (g o) -> p g o", o=1),
                    act_raw[:].rearrange("p (g k) -> p g k", g=GRP),
                    axis=AX.X)
                den = aux.tile([128, GRP], F32, tag="den")
                nc.vector.tensor_mul(den[:], att[:], sums[:])
                scl = aux.tile([128, GRP], F32, tag="scl")
                nc.vector.reciprocal(scl[:], den[:])
                nc.vector.tensor_scalar_mul(scl[:], scl[:], rmask_sb[:])
                state[rt] = (h_sb, act_raw, scl)

            def vlad_phase(rt):
                h_sb, act_raw, scl = state.pop(rt)
                bg = rt * 2
                pv = pvpool.tile([128, FS], F32, tag="pv", bufs=1)
                s128 = pvpool.tile([128, 1], F32, tag="s128", bufs=1)
                for g in range(GRP):
                    bd = bd_tiles[g % 3]
                    nc.vector.tensor_scalar_mul(
                        bd[0:64, 0:64],
                        act_raw[0:64, g * 64:(g + 1) * 64],
                        scl[0:64, g:g + 1])
                    nc.vector.tensor_scalar_mul(
                        bd[64:128, 64:128],
                        act_raw[64:128, g * 64:(g + 1) * 64],
                        scl[64:128, g:g + 1])
                    nc.tensor.matmul(pv[:], bd[:],
                                     h_sb[:, g * FS:(g + 1) * FS],
                                     start=(g == 0), stop=(g == GRP - 1))
                    if g == 0:
                        nc.vector.tensor_copy(bdsum[:], bd[:])
                    else:
                        nc.vector.tensor_add(bdsum[:], bdsum[:], bd[:])
                nc.tensor.matmul(s128[:], bdsum[:], onesb[:],
                                 start=True, stop=True,
                                 skip_group_check=True)
                s_sb = aux.tile([128, 1], F32, tag="s_sb")
                nc.vector.tensor_copy(s_sb[:], s128[:])
                tmp = vopool.tile([128, FS], F32, tag="tmpc2")
                nc.vector.tensor_scalar_mul(tmp[:], c2t2_sb[:], s_sb[:, 0:1])
                vout = vopool.tile([128, FS], F16, tag="vout")
                nc.vector.tensor_sub(vout[:], pv[:], tmp[:])

                # scatter k-slices to the a2a input, pre-blocked by
                # destination core (sync queue; xk all pre-issued).
                vdst = vlA if rt < 4 else vlB
                ib = bg if rt < 4 else bg - 8
                for c in range(NCORES):
                    for b2 in range(2):
                        r = c * 8 + ib + b2
                        nc.sync.dma_start(
                            vdst[r:r + 1, :].rearrange(
                                "o (kk f) -> (o kk) f", kk=KPC),
                            vout[b2 * 64 + c * KPC:
                                 b2 * 64 + (c + 1) * KPC, :])

            # software-pipelined trace: h(rt+1) is emitted before
            # vlad(rt) so the in-order PE queue never waits on the
            # softmax chain.
            for rt in range(RT):
                h_phase(rt)
                if rt >= 1:
                    vlad_phase(rt - 1)
                if rt == 4 and collectives:
                    # vlA complete after vlad_phase(3); trigger hidden
                    nc.gpsimd.collective_compute(
                        "AllToAll", ALU.bypass, replica_groups=RG,
                        ins=[vlA.opt()], outs=[slcA.opt()])
            vlad_phase(RT - 1)
            if not collectives:
                nc.sync.dma_start(slcA[:, :], vlA[:, :])

            if collectives:
                nc.gpsimd.collective_compute(
                    "AllToAll", ALU.bypass, replica_groups=RG,
                    ins=[vlB.opt()], outs=[slcB.opt()])
            else:
                nc.sync.dma_start(slcB[:, :], vlB[:, :])

        # ================= context gating =================
        with tc.tile_pool(name="cgv", bufs=3) as vpool, \
             tc.tile_pool(name="cgsb", bufs=2) as cgsb, \
             tc.tile_pool(name="cgp", bufs=1, space="PSUM") as cgps, \
             tc.tile_pool(name="cgpt", bufs=2, space="PSUM") as cgpt:
            py = [cgps.tile([128, 512], F32, tag=f"py{i}", name=f"py{i}",
                            bufs=1) for i in range(4)]
            for kt in range(NKT):
                wch = wpool.tile([128, HID], BF16, tag="wch", name="wch",
                                 bufs=12)
                nc.sync.dma_start(wch[:], wcg[kt * 128:(kt + 1) * 128, :])
                vload = vpool.tile([128, 128], BF16, tag="vload",
                                   name="vload", bufs=3)
                nc.sync.dma_start(vload[0:64, :],
                                  slcA[:, kt * 128:(kt + 1) * 128])
                nc.sync.dma_start(vload[64:128, :],
                                  slcB[:, kt * 128:(kt + 1) * 128])
                pt = cgpt.tile([128, 128], BF16, tag="pt", bufs=2)
                nc.tensor.transpose(pt[:], vload[:], identb[:])
                vt = vpool.tile([128, 128], BF16, tag="vt", name="vt",
                                bufs=3)
                nc.vector.tensor_copy(vt[:], pt[:])
                for ch in range(4):
                    nc.tensor.matmul(py[ch][:], vt[:],
                                     wch[:, ch * 512:(ch + 1) * 512],
                                     start=(kt == 0), stop=(kt == NKT - 1),
                                     skip_group_check=(ch > 0))
            for ch in range(4):
                ych = cgsb.tile([128, 512], F16, tag="ych")
                nc.vector.tensor_copy(ych[:], py[ch][:])
                nc.sync.dma_start(ypart[:, ch * 512:(ch + 1) * 512], ych[:])

        if collectives:
            nc.gpsimd.collective_compute(
                "AllReduce", ALU.add, replica_groups=RG,
                ins=[ypart.opt()], outs=[y_all.opt()])
        else:
            nc.sync.dma_start(y_all[:, :], ypart[:, :])

        # ================= replicated tail =================
        with tc.tile_pool(name="tsb", bufs=1) as tsb, \
             tc.tile_pool(name="taux", bufs=2) as taux, \
             tc.tile_pool(name="tps", bufs=1, space="PSUM") as tps, \
             tc.tile_pool(name="tpt", bufs=2, space="PSUM") as tpt:
            y_sb = tsb.tile([128, HID], F16, tag="y_sb")
            nc.sync.dma_start(y_sb[:], y_all[:])
            sq = tsb.tile([128, HID], F16, tag="sq")
            for ch in range(4):
                nc.vector.tensor_mul(sq[:, ch * 512:(ch + 1) * 512],
                                     y_sb[:, ch * 512:(ch + 1) * 512],
                                     y_sb[:, ch * 512:(ch + 1) * 512])

            # per-column stats via N=1 matmuls with a ones vector:
            # pstat[:, ct] = sum_b y[b, ct*128+p], [:, 16+ct] = sum y^2
            pstat = tps.tile([128, 32], F32, tag="pstat", bufs=1)
            for ct in range(16):
                nc.tensor.matmul(pstat[:, ct:ct + 1],
                                 y_sb[:, ct * 128:(ct + 1) * 128], onesb[:],
                                 start=True, stop=True,
                                 skip_group_check=(ct > 0))
            for ct in range(16):
                nc.tensor.matmul(pstat[:, 16 + ct:17 + ct],
                                 sq[:, ct * 128:(ct + 1) * 128], onesb[:],
                                 start=True, stop=True,
                                 skip_group_check=True)
            mu = taux.tile([128, 16], F32, tag="mu")
            nc.vector.tensor_scalar_mul(mu[:], pstat[:, 0:16], 1.0 / B)
            ex2 = taux.tile([128, 16], F32, tag="ex2")
            nc.vector.tensor_scalar_mul(ex2[:], pstat[:, 16:32], 1.0 / B)
            musq = taux.tile([128, 16], F32, tag="musq")
            nc.vector.tensor_mul(musq[:], mu[:], mu[:])
            var = taux.tile([128, 16], F32, tag="var")
            nc.vector.tensor_sub(var[:], ex2[:], musq[:])
            sd = taux.tile([128, 16], F32, tag="sd")
            nc.scalar.activation(sd[:], var[:], AF.Sqrt, bias=eps1[:, 0:1])
            rstd = taux.tile([128, 16], F32, tag="rstd")
            nc.vector.reciprocal(rstd[:], sd[:])
            seff = taux.tile([128, 16], F32, tag="seff")
            nc.vector.tensor_mul(seff[:], bn1gT_sb[:], rstd[:])
            mue = taux.tile([128, 16], F32, tag="mue")
            nc.vector.tensor_mul(mue[:], mu[:], seff[:])
            beff = taux.tile([128, 16], F32, tag="beff")
            nc.vector.tensor_sub(beff[:], bn1bT_sb[:], mue[:])

            # transpose y tile-by-tile and normalize: ybnT[ct] is
            # [128 cols, 128 imgs] bf16
            ybnT = []
            for ct in range(16):
                ptr = tpt.tile([128, 128], F16, tag="ptr", bufs=2)
                nc.tensor.transpose(ptr[:], y_sb[:, ct * 128:(ct + 1) * 128],
                                    identb[:])
                yt = tsb.tile([128, 128], BF16, tag=f"ybnT{ct}",
                              name=f"ybnT{ct}")
                nc.vector.tensor_scalar(yt[:], ptr[:], seff[:, ct:ct + 1],
                                        beff[:, ct:ct + 1], ALU.mult,
                                        ALU.add)
                ybnT.append(yt)

            # z = ybn @ Wg1  [128 imgs, 256]
            pz = tps.tile([128, RED], F32, tag="pz", bufs=1)
            for ct in range(16):
                nc.tensor.matmul(pz[:], ybnT[ct][:],
                                 wg1_sb[:, ct * RED:(ct + 1) * RED],
                                 start=(ct == 0), stop=(ct == 15))
            z_sb = tsb.tile([128, RED], F16, tag="z_sb")
            nc.vector.tensor_copy(z_sb[:], pz[:])
            sq2 = tsb.tile([128, RED], F16, tag="sq2")
            nc.vector.tensor_mul(sq2[:], z_sb[:], z_sb[:])
            pstat2 = tps.tile([128, 4], F32, tag="pstat2", bufs=1)
            for j in range(2):
                nc.tensor.matmul(pstat2[:, j:j + 1],
                                 z_sb[:, j * 128:(j + 1) * 128], onesb[:],
                                 start=True, stop=True,
                                 skip_group_check=(j > 0))
                nc.tensor.matmul(pstat2[:, 2 + j:3 + j],
                                 sq2[:, j * 128:(j + 1) * 128], onesb[:],
                                 start=True, stop=True,
                                 skip_group_check=True)
            mu2 = taux.tile([128, 2], F32, tag="mu2")
            nc.vector.tensor_scalar_mul(mu2[:], pstat2[:, 0:2], 1.0 / B)
            ex22 = taux.tile([128, 2], F32, tag="ex22")
            nc.vector.tensor_scalar_mul(ex22[:], pstat2[:, 2:4], 1.0 / B)
            musq2 = taux.tile([128, 2], F32, tag="musq2")
            nc.vector.tensor_mul(musq2[:], mu2[:], mu2[:])
            var2 = taux.tile([128, 2], F32, tag="var2")
            nc.vector.tensor_sub(var2[:], ex22[:], musq2[:])
            sd2 = taux.tile([128, 2], F32, tag="sd2")
            nc.scalar.activation(sd2[:], var2[:], AF.Sqrt, bias=eps1[:, 0:1])
            rstd2 = taux.tile([128, 2], F32, tag="rstd2")
            nc.vector.reciprocal(rstd2[:], sd2[:])
            seff2 = taux.tile([128, 2], F32, tag="seff2")
            nc.vector.tensor_mul(seff2[:], bn2gT_sb[:], rstd2[:])
            mue2 = taux.tile([128, 2], F32, tag="mue2")
            nc.vector.tensor_mul(mue2[:], mu2[:], seff2[:])
            beff2 = taux.tile([128, 2], F32, tag="beff2")
            nc.vector.tensor_sub(beff2[:], bn2bT_sb[:], mue2[:])

            rT = []
            for j in range(2):
                ptr = tpt.tile([128, 128], F16, tag="ptr", bufs=2)
                nc.tensor.transpose(ptr[:], z_sb[:, j * 128:(j + 1) * 128],
                                    identb[:])
                tz = taux.tile([128, 128], F32, tag="tz")
                nc.vector.tensor_scalar(tz[:], ptr[:], seff2[:, j:j + 1],
                                        beff2[:, j:j + 1], ALU.mult,
                                        ALU.add)
                rt_ = tsb.tile([128, 128], BF16, tag=f"rT{j}", name=f"rT{j}")
                nc.vector.tensor_scalar_max(rt_[:], tz[:], 0.0)
                rT.append(rt_)

            # gateT = sigmoid(Wg2^T @ r^T + bg2), oT = ybnT * gateT,
            # outT = W3^T @ oT  -- all in the transposed layout.
            po = tps.tile([NCLS, B], F32, tag="po", bufs=1)
            for m in range(16):
                pg = tpt.tile([128, 128], F32, tag="pg", bufs=2)
                for j in range(2):
                    nc.tensor.matmul(pg[:], wg2_sb[j][:, m * 128:(m + 1) * 128],
                                     rT[j][:], start=(j == 0), stop=(j == 1))
                gt = taux.tile([128, 128], BF16, tag="gt")
                nc.scalar.activation(gt[:], pg[:], AF.Sigmoid,
                                     bias=bg2T_sb[:, m:m + 1])
                ot = taux.tile([128, 128], BF16, tag="ot")
                nc.vector.tensor_mul(ot[:], ybnT[m][:], gt[:])
                nc.tensor.matmul(po[:], w3_sb[:, m * NCLS:(m + 1) * NCLS],
                                 ot[:], start=(m == 0), stop=(m == 15),
                                 skip_group_check=True)
            out_sb = taux.tile([NCLS, B], F32, tag="outp")
            nc.vector.tensor_copy(out_sb[:], po[:])
            nc.sync.dma_start(outT[:], out_sb[:])

    nc.compile()
    return nc


def _host_prep(inputs):
    f32 = np.float32
    bf16 = np.float16
    g = {k: np.asarray(v, dtype=f32) for k, v in inputs.items()}

    x2 = np.transpose(g["x"], (0, 3, 1, 2)).reshape(B, T, POSE)
    Wh = g["W_emb"] @ g["W1"]                       # [144, 4096]
    bh = g["b_emb"] @ g["W1"] + g["b1"]             # [4096]
    C1cat = np.concatenate([g["centroids1"], g["W2"]], axis=1)  # [4096, 520]
    WL = Wh @ C1cat                                 # [144, 520]
    bL = bh @ C1cat
    bL[FS:] += g["b2"]
    wh_aug = np.concatenate([Wh, bh[None, :]], axis=0).astype(bf16)
    wl_aug = np.concatenate([WL, bL[None, :]], axis=0).astype(bf16)
    c2t = np.ascontiguousarray(g["centroids2"][0].T)            # [64, 512]
    c2t2 = np.concatenate([c2t, c2t], axis=0)                   # [128, 512]

    # permute Wcg rows: our flat vlad index is k*FS+f, reference is f*K+k
    new = np.arange(VLAD)
    old = (new % FS) * K + (new // FS)
    wcg_perm = g["Wcg"][old, :].astype(bf16)        # [32768, 2048]

    rmask = np.zeros((128, 1), f32)
    rmask[:T] = 1.0
    rmask[TP:TP + T] = 1.0
    identb_np = np.eye(128, dtype=np.float16)
    identf_np = np.eye(128, dtype=f32)
    onesb_np = np.ones((128, 1), np.float16)
    onesf_np = np.ones((128, 1), f32)
    common = {
        "wh": wh_aug,
        "wl": wl_aug,
        "c2t2": c2t2,
        "wg1": g["Wg1"].astype(bf16),
        "wg2": g["Wg2"].astype(bf16),
        "w3": g["W3"].astype(bf16),
        "bn1gT": np.ascontiguousarray(g["g_bn1"].reshape(16, 128).T),
        "bn1bT": np.ascontiguousarray(g["b_bn1"].reshape(16, 128).T),
        "bn2gT": np.ascontiguousarray(g["g_bn2"].reshape(2, 128).T),
        "bn2bT": np.ascontiguousarray(g["b_bn2"].reshape(2, 128).T),
        "bg2T": np.ascontiguousarray(g["bg2"].reshape(16, 128).T),
        "rowmask": rmask,
        "identb_in": identb_np,
        "identf_in": identf_np,
        "onesb_in": onesb_np,
        "onesf_in": onesf_np,
    }
    in_maps = []
    for c in range(NCORES):
        xs = x2[c * BPC:(c + 1) * BPC]              # [16, 60, 144]
        xp = np.zeros((BPC, TP, POSE), f32)
        xp[:, :T] = xs
        xTf = np.ones((KA, ROWS), f32)
        xTf[:POSE] = xp.reshape(ROWS, POSE).T
        m = dict(common)
        m["xT"] = xTf.astype(bf16)
        m["wcg"] = np.ascontiguousarray(wcg_perm[c * SLC:(c + 1) * SLC, :])
        in_maps.append(m)
    return in_maps, g["b3"]


# vstage row r <-> global batch index (r<64: core r//8 img r%8,
# r>=64: core (r-64)//8 img 8+(r-64)%8)
_PERM = np.zeros(B, np.int64)
for _r in range(B):
    if _r < 64:
        _s, _i = divmod(_r, 8)
        _PERM[_r] = BPC * _s + _i
    else:
        _s, _i = divmod(_r - 64, 8)
        _PERM[_r] = BPC * _s + 8 + _i


def kernel(**inputs):
    if "nc" not in _CACHED:
        _CACHED["nc"] = _build_nc()
    nc = _CACHED["nc"]
    in_maps, b3 = _host_prep(inputs)
    res = bass_utils.run_bass_kernel_spmd(nc, in_maps,
                                          core_ids=list(range(NCORES)))
    _CACHED["last_res"] = res
    out = np.zeros((B, NCLS), np.float32)
    out[_PERM, :] = res.results[0]["outT"].T
    return out + b3[None, :]


# revision 46
# speedup vs baseline: 1.0369x; 1.0198x over previous
"""NextVLAD Trainium2 kernel: 8-core SPMD bass/tile implementation (v3).

Strategy:
  * Host folds W_emb@W1 (and the centroid/attention projections) so the
    front end contracts over K=145 instead of K=2048.  All matmuls run
    in fp16 (full PE rate like bf16, but 8x less quantization noise;
    fp32 runs 2-4x slower on the PE).  Only the softmax exp output
    stays f32 (range).
  * Front end is data-parallel over batch (16 images/core, rows padded
    to 64 per image).  The VLAD einsum uses a block-diagonal act tile
    so two images share one 512-col PE stream; the act row-sum needed
    for the centroid term rides on a DVE-accumulated bd sum.
  * The trace is software-pipelined (h-phase of row-tile rt is emitted
    before the vlad-phase of rt-1) because engine queues execute in
    order: the PE must never sit behind the softmax dependency chain.
    DMA issue queues are load-balanced (sync: x/wh/scatters, scalar:
    second weight half + tail weights late, gpsimd: paced wcg stream)
    since a DMA issue that blocks mid-queue stalls everything behind it.
  * Each core writes its VLAD output pre-blocked by destination core;
    two AllToAll collectives (images 0-7 hidden under the front end,
    8-15 after) give every core the full batch for its own 4096-wide
    k-slice of the cg.fc1 contraction -- 1MB moved instead of an 8MB
    AllGather, and 32 PE transposes instead of 256.
  * Context gating: y_partial[128,2048] = vlads_slice^T @ wcg_slice,
    wcg (16.8MB fp16) fully prefetched into SBUF during the front end.
    One fp16 AllReduce combines the k-slice partials; the small tail
    (BN1, gating, fc2, logits) is computed replicated on every core in
    a transposed layout (stats via ones-vector matmuls), so no further
    collectives are needed.  BN invariances let bcg/bg1 be dropped and
    all BN scales folded on-chip.
"""

import numpy as np
import ml_dtypes

import concourse.bass as bass
import concourse.mybir as mybir
import concourse.tile as tile
from concourse import bacc, bass_utils

F32 = mybir.dt.float32
F32R = mybir.dt.float32r
BF16 = mybir.dt.bfloat16

B, T, POSE = 128, 60, 144
DIM, EXP, GRP, K, NCLS = 2048, 2, 8, 64, 10
ED = EXP * DIM            # 4096
FS = ED // GRP            # 512
VLAD = K * FS             # 32768
HID = DIM                 # 2048
RED = HID // 8            # 256

NCORES = 8
BPC = B // NCORES         # 16 images per core
TP = 64                   # padded rows per image (60 real + 4 pad)
ROWS = BPC * TP           # 1024 rows per core
RT = ROWS // 128          # 8 row tiles (2 images each)
KA = POSE + 1             # 145 contraction (with bias row)
SLC = VLAD // NCORES      # 4096 k-slice columns per core
KPC = K // NCORES         # 8 clusters per core slice
NKT = SLC // 128          # 32 k-tiles in the cg contraction
EPS = 1e-5

_CACHED = {}


def _build_nc(collectives=True):
    nc = bacc.Bacc("TRN2", target_bir_lowering=False, debug=False,
                   num_devices=NCORES)

    xT = nc.dram_tensor("xT", [KA, ROWS], BF16, kind="ExternalInput").ap()
    wh = nc.dram_tensor("wh", [KA, ED], BF16, kind="ExternalInput").ap()
    wl = nc.dram_tensor("wl", [KA, FS + GRP], BF16, kind="ExternalInput").ap()
    c2t2 = nc.dram_tensor("c2t2", [128, FS], F32, kind="ExternalInput").ap()
    wcg = nc.dram_tensor("wcg", [SLC, HID], BF16, kind="ExternalInput").ap()
    wg1 = nc.dram_tensor("wg1", [HID, RED], BF16, kind="ExternalInput").ap()
    wg2 = nc.dram_tensor("wg2", [RED, HID], BF16, kind="ExternalInput").ap()
    w3 = nc.dram_tensor("w3", [HID, NCLS], BF16, kind="ExternalInput").ap()
    bn1gT = nc.dram_tensor("bn1gT", [128, 16], F32, kind="ExternalInput").ap()
    bn1bT = nc.dram_tensor("bn1bT", [128, 16], F32, kind="ExternalInput").ap()
    bn2gT = nc.dram_tensor("bn2gT", [128, 2], F32, kind="ExternalInput").ap()
    bn2bT = nc.dram_tensor("bn2bT", [128, 2], F32, kind="ExternalInput").ap()
    bg2T = nc.dram_tensor("bg2T", [128, 16], F32, kind="ExternalInput").ap()
    rowmask = nc.dram_tensor("rowmask", [128, 1], F32, kind="ExternalInput").ap()
    identb_in = nc.dram_tensor("identb_in", [128, 128], BF16,
                               kind="ExternalInput").ap()
    identf_in = nc.dram_tensor("identf_in", [128, 128], F32,
                               kind="ExternalInput").ap()
    onesb_in = nc.dram_tensor("onesb_in", [128, 1], BF16,
                              kind="ExternalInput").ap()
    onesf_in = nc.dram_tensor("onesf_in", [128, 1], F32,
                              kind="ExternalInput").ap()
    outT = nc.dram_tensor("outT", [NCLS, B], F32, kind="ExternalOutput").ap()

    AF = mybir.ActivationFunctionType
    AX = mybir.AxisListType
    ALU = mybir.AluOpType
    RG = [list(range(NCORES))]

    with tile.TileContext(nc) as tc:
      with tc.tile_pool(name="const", bufs=1) as const, \
           tc.tile_pool(name="wstream", bufs=12) as wpool, \
           tc.tile_pool(name="dram", bufs=1, space="DRAM") as dram:
        # ---- constants loaded once ----
        wh0 = const.tile([128, ED], BF16)
        wh1 = const.tile([KA - 128, ED], BF16)
        nc.sync.dma_start(wh0[:], wh[0:128, :])
        nc.sync.dma_start(wh1[:], wh[128:KA, :])
        wl0 = const.tile([128, FS + GRP], BF16)
        wl1 = const.tile([KA - 128, FS + GRP], BF16)
        nc.scalar.dma_start(wl0[:], wl[0:128, :])
        nc.scalar.dma_start(wl1[:], wl[128:KA, :])
        c2t2_sb = const.tile([128, FS], F32)
        nc.scalar.dma_start(c2t2_sb[:], c2t2[:])
        rmask_sb = const.tile([128, 1], F32)
        nc.scalar.dma_start(rmask_sb[:], rowmask[:])
        identb = const.tile([128, 128], BF16)
        nc.scalar.dma_start(identb[:], identb_in[:])
        onesb = const.tile([128, 1], BF16)
        nc.scalar.dma_start(onesb[:], onesb_in[:])
        onesf = const.tile([128, 1], F32)
        nc.scalar.dma_start(onesf[:], onesf_in[:])
        eps1 = const.tile([128, 1], F32)
        nc.any.memset(eps1[:, :], EPS)

        # tail weights: wg1 as rhs tiles [128k, 256], wg2 as lhsT rows,
        # w3 as lhsT tiles [128k, 10]
        wg1_sb = const.tile([128, 16 * RED], BF16)
        nc.scalar.dma_start(wg1_sb[:].rearrange("p (kt n) -> p kt n", kt=16),
                            wg1[:].rearrange("(kt p) n -> p kt n", p=128))
        wg2_sb = []
        for kt in range(2):
            t = const.tile([128, HID], BF16, tag=f"wg2_{kt}")
            nc.scalar.dma_start(t[:], wg2[kt * 128:(kt + 1) * 128, :])
            wg2_sb.append(t)
        w3_sb = const.tile([128, 16 * NCLS], BF16)
        nc.scalar.dma_start(w3_sb[:].rearrange("p (kt n) -> p kt n", kt=16),
                            w3[:].rearrange("(kt p) n -> p kt n", p=128))

        # block-diagonal act tiles (off-diagonal stays zero forever)
        bd_tiles = []
        for i in range(3):
            t = const.tile([128, 128], BF16, tag=f"bd{i}")
            nc.any.memset(t[:, :], 0.0)
            bd_tiles.append(t)

        # dram scratch
        vlA = dram.tile([64, SLC], BF16)
        vlB = dram.tile([64, SLC], BF16)
        shared = "Shared" if collectives else "Local"
        slcA = dram.tile([64, SLC], BF16)
        slcB = dram.tile([64, SLC], BF16)
        ypart = dram.tile([B, HID], F16)
        y_all = dram.tile([B, HID], F16, addr_space=shared)

        # ================= front end =================
        with tc.tile_pool(name="fex", bufs=3) as xpool, \
             tc.tile_pool(name="feh", bufs=2) as hpool, \
             tc.tile_pool(name="feact", bufs=2) as apool, \
             tc.tile_pool(name="feaux", bufs=3) as aux, \
             tc.tile_pool(name="fevo", bufs=2) as vopool, \
             tc.tile_pool(name="ph", bufs=3, space="PSUM") as phpool, \
             tc.tile_pool(name="pl", bufs=2, space="PSUM") as plpool, \
             tc.tile_pool(name="pv", bufs=2, space="PSUM") as pvpool:
            wch_tiles = []
            _pace = [0, 6, 6, 6, 0, 5, 5, 4]
            # preload every row-tile's x up front (tiny) so no later
            # sync-queue stall (scatters block on compute) can starve the
            # h matmuls of weights.
            xk_tiles = {}
            for rt in range(RT):
                rs = rt * 128
                xk0 = xpool.tile([128, 128], F16, tag=f"xk0_{rt}", bufs=1)
                xk1 = xpool.tile([KA - 128, 128], F16, tag=f"xk1_{rt}",
                                 bufs=1)
                eng = nc.sync if rt < 4 else nc.scalar
                eng.dma_start(xk0[:], xT[0:128, rs:rs + 128])
                eng.dma_start(xk1[:], xT[128:KA, rs:rs + 128])
                xk_tiles[rt] = (xk0, xk1)
            state = {}

            def h_phase(rt):
                # paced wcg prefetch (gpsimd queue, separate hw queue from
                # the sync-queue scatters)
                for _q in range(_pace[rt]):
                    kt = len(wch_tiles)
                    wch = wpool.tile([128, HID], F16, tag=f"wch{kt}",
                                     name=f"wch{kt}", bufs=1)
                    nc.gpsimd.dma_start(wch[:],
                                        wcg[kt * 128:(kt + 1) * 128, :])
                    wch_tiles.append(wch)
                xk0, xk1 = xk_tiles.pop(rt)
                h_sb = hpool.tile([128, ED], F16, tag="h", bufs=3)
                for nt in range(ED // 1024):
                    # 1024-wide chunk spans two PSUM banks; each matmul dst
                    # stays inside one bank.  Halves the copy count and the
                    # PE<->copy semaphore round-trips that pace this loop.
                    ph = phpool.tile([128, 1024], F32, tag="ph", bufs=2)
                    for half in range(2):
                        cs = nt * 1024 + half * 512
                        nc.tensor.matmul(ph[:, half * 512:(half + 1) * 512],
                                         xk0[:],
                                         wh0q[nt][:, half * 512:
                                                  (half + 1) * 512],
                                         start=True, stop=False,
                                         skip_group_check=(half > 0))
                        nc.tensor.matmul(ph[:, half * 512:(half + 1) * 512],
                                         xk1[:], wh1[:, cs:cs + 512],
                                         start=False, stop=True,
                                         skip_group_check=(half > 0))
                    if nt in (1, 3):
                        nc.vector.tensor_copy(
                            h_sb[:, nt * 1024:(nt + 1) * 1024], ph[:])
                    else:
                        nc.scalar.copy(h_sb[:, nt * 1024:(nt + 1) * 1024],
                                       ph[:])
                pl = plpool.tile([128, 512], F32, tag="pl", bufs=1)
                nc.tensor.matmul(pl[:], xk0[:], wl0[:, 0:512],
                                 start=True, stop=False)
                nc.tensor.matmul(pl[:], xk1[:], wl1[:, 0:512],
                                 start=False, stop=True)
                pa = plpool.tile([128, GRP], F32, tag="pa", bufs=1)
                nc.tensor.matmul(pa[:], xk0[:], wl0[:, 512:512 + GRP],
                                 start=True, stop=False)
                nc.tensor.matmul(pa[:], xk1[:], wl1[:, 512:512 + GRP],
                                 start=False, stop=True)

                # softmax without max-subtraction: logits are O(30) so
                # exp() stays well inside f32 range.
                act_raw = apool.tile([128, 512], F32, tag="act", bufs=2)
                nc.scalar.activation(act_raw[:], pl[:], AF.Exp)
                att = aux.tile([128, GRP], F32, tag="att")
                nc.scalar.activation(att[:], pa[:], AF.Exp, scale=-1.0)
                nc.vector.tensor_scalar_add(att[:], att[:], 1.0)
                sums = aux.tile([128, GRP], F32, tag="sums")
                nc.vector.reduce_sum(
                    sums[:].rearrange("p (g o) -> p g o", o=1),
                    act_raw[:].rearrange("p (g k) -> p g k", g=GRP),
                    axis=AX.X)
                den = aux.tile([128, GRP], F32, tag="den")
                nc.vector.tensor_mul(den[:], att[:], sums[:])
                scl = aux.tile([128, GRP], F32, tag="scl")
                nc.vector.reciprocal(scl[:], den[:])
                nc.vector.tensor_scalar_mul(scl[:], scl[:], rmask_sb[:])
                state[rt] = (h_sb, act_raw, scl)

            def vlad_phase(rt):
                h_sb, act_raw, scl = state.pop(rt)
                bg = rt * 2
                pv = pvpool.tile([128, FS], F32, tag="pv", bufs=1)
                s128 = pvpool.tile([128, 1], F32, tag="s128", bufs=1)
                for g in range(GRP):
                    bd = bd_tiles[g % 3]
                    nc.vector.tensor_scalar_mul(
                        bd[0:64, 0:64],
                        act_raw[0:64, g * 64:(g + 1) * 64],
                        scl[0:64, g:g + 1])
                    nc.vector.tensor_scalar_mul(
                        bd[64:128, 64:128],
                        act_raw[64:128, g * 64:(g + 1) * 64],
                        scl[64:128, g:g + 1])
                    nc.tensor.matmul(pv[:], bd[:],
                                     h_sb[:, g * FS:(g + 1) * FS],
                                     start=(g == 0), stop=(g == GRP - 1))
                    if g == 0:
                        nc.vector.tensor_copy(bdsum[:], bd[:])
                    else:
                        nc.vector.tensor_add(bdsum[:], bdsum[:], bd[:])
                nc.tensor.matmul(s128[:], bdsum[:], onesb[:],
                                 start=True, stop=True,
                                 skip_group_check=True)
                s_sb = aux.tile([128, 1], F32, tag="s_sb")
                nc.vector.tensor_copy(s_sb[:], s128[:])
                tmp = vopool.tile([128, FS], F32, tag="tmpc2")
                nc.vector.tensor_scalar_mul(tmp[:], c2t2_sb[:], s_sb[:, 0:1])
                vout = vopool.tile([128, FS], F16, tag="vout")
                nc.vector.tensor_sub(vout[:], pv[:], tmp[:])

                # scatter k-slices to the a2a input, pre-blocked by
                # destination core (sync queue; xk all pre-issued).
                vdst = vlA if rt < 4 else vlB
                ib = bg if rt < 4 else bg - 8
                for c in range(NCORES):
                    for b2 in range(2):
                        r = c * 8 + ib + b2
                        nc.sync.dma_start(
                            vdst[r:r + 1, :].rearrange(
                                "o (kk f) -> (o kk) f", kk=KPC),
                            vout[b2 * 64 + c * KPC:
                                 b2 * 64 + (c + 1) * KPC, :])

            # software-pipelined trace: h(rt+1) is emitted before
            # vlad(rt) so the in-order PE queue never waits on the
            # softmax chain.
            for rt in range(RT):
                h_phase(rt)
                if rt >= 1:
                    vlad_phase(rt - 1)
                if rt == 4 and collectives:
                    # vlA complete after vlad_phase(3); trigger hidden
                    nc.gpsimd.collective_compute(
                        "AllToAll", ALU.bypass, replica_groups=RG,
                        ins=[vlA.opt()], outs=[slcA.opt()])
            vlad_phase(RT - 1)
            if not collectives:
                nc.sync.dma_start(slcA[:, :], vlA[:, :])

            if collectives:
                nc.gpsimd.collective_compute(
                    "AllToAll", ALU.bypass, replica_groups=RG,
                    ins=[vlB.opt()], outs=[slcB.opt()])
            else:
                nc.sync.dma_start(slcB[:, :], vlB[:, :])

        # ================= context gating =================
        with tc.tile_pool(name="cgv", bufs=3) as vpool, \
             tc.tile_pool(name="cgsb", bufs=2) as cgsb, \
             tc.tile_pool(name="cgp", bufs=1, space="PSUM") as cgps, \
             tc.tile_pool(name="cgpt", bufs=2, space="PSUM") as cgpt:
            py = [cgps.tile([128, 512], F32, tag=f"py{i}", name=f"py{i}",
                            bufs=1) for i in range(4)]
            for kt in range(NKT):
                wch = wpool.tile([128, HID], BF16, tag="wch", name="wch",
                                 bufs=12)
                nc.sync.dma_start(wch[:], wcg[kt * 128:(kt + 1) * 128, :])
                vload = vpool.tile([128, 128], BF16, tag="vload",
                                   name="vload", bufs=3)
                nc.sync.dma_start(vload[0:64, :],
                                  slcA[:, kt * 128:(kt + 1) * 128])
                nc.sync.dma_start(vload[64:128, :],
                                  slcB[:, kt * 128:(kt + 1) * 128])
                pt = cgpt.tile([128, 128], BF16, tag="pt", bufs=2)
                nc.tensor.transpose(pt[:], vload[:], identb[:])
                vt = vpool.tile([128, 128], BF16, tag="vt", name="vt",
                                bufs=3)
                nc.vector.tensor_copy(vt[:], pt[:])
                for ch in range(4):
                    nc.tensor.matmul(py[ch][:], vt[:],
                                     wch[:, ch * 512:(ch + 1) * 512],
                                     start=(kt == 0), stop=(kt == NKT - 1),
                                     skip_group_check=(ch > 0))
            for ch in range(4):
                ych = cgsb.tile([128, 512], F16, tag="ych")
                nc.vector.tensor_copy(ych[:], py[ch][:])
                nc.sync.dma_start(ypart[:, ch * 512:(ch + 1) * 512], ych[:])

        if collectives:
            nc.gpsimd.collective_compute(
                "AllReduce", ALU.add, replica_groups=RG,
                ins=[ypart.opt()], outs=[y_all.opt()])
        else:
            nc.sync.dma_start(y_all[:, :], ypart[:, :])

        # ================= replicated tail =================
        with tc.tile_pool(name="tsb", bufs=1) as tsb, \
             tc.tile_pool(name="taux", bufs=2) as taux, \
             tc.tile_pool(name="tps", bufs=1, space="PSUM") as tps, \
             tc.tile_pool(name="tpt", bufs=2, space="PSUM") as tpt:
            y_sb = tsb.tile([128, HID], F16, tag="y_sb")
            nc.sync.dma_start(y_sb[:], y_all[:])
            sq = tsb.tile([128, HID], F16, tag="sq")
            for ch in range(4):
                nc.vector.tensor_mul(sq[:, ch * 512:(ch + 1) * 512],
                                     y_sb[:, ch * 512:(ch + 1) * 512],
                                     y_sb[:, ch * 512:(ch + 1) * 512])

            # per-column stats via N=1 matmuls with a ones vector:
            # pstat[:, ct] = sum_b y[b, ct*128+p], [:, 16+ct] = sum y^2
            pstat = tps.tile([128, 32], F32, tag="pstat", bufs=1)
            for ct in range(16):
                nc.tensor.matmul(pstat[:, ct:ct + 1],
                                 y_sb[:, ct * 128:(ct + 1) * 128], onesb[:],
                                 start=True, stop=True,
                                 skip_group_check=(ct > 0))
            for ct in range(16):
                nc.tensor.matmul(pstat[:, 16 + ct:17 + ct],
                                 sq[:, ct * 128:(ct + 1) * 128], onesb[:],
                                 start=True, stop=True,
                                 skip_group_check=True)
            mu = taux.tile([128, 16], F32, tag="mu")
            nc.vector.tensor_scalar_mul(mu[:], pstat[:, 0:16], 1.0 / B)
            ex2 = taux.tile([128, 16], F32, tag="ex2")
            nc.vector.tensor_scalar_mul(ex2[:], pstat[:, 16:32], 1.0 / B)
            musq = taux.tile([128, 16], F32, tag="musq")
            nc.vector.tensor_mul(musq[:], mu[:], mu[:])
            var = taux.tile([128, 16], F32, tag="var")
            nc.vector.tensor_sub(var[:], ex2[:], musq[:])
            sd = taux.tile([128, 16], F32, tag="sd")
            nc.scalar.activation(sd[:], var[:], AF.Sqrt, bias=eps1[:, 0:1])
            rstd = taux.tile([128, 16], F32, tag="rstd")
            nc.vector.reciprocal(rstd[:], sd[:])
            seff = taux.tile([128, 16], F32, tag="seff")
            nc.vector.tensor_mul(seff[:], bn1gT_sb[:], rstd[:])
            mue = taux.tile([128, 16], F32, tag="mue")
            nc.vector.tensor_mul(mue[:], mu[:], seff[:])
            beff = taux.tile([128, 16], F32, tag="beff")
            nc.vector.tensor_sub(beff[:], bn1bT_sb[:], mue[:])

            # transpose y tile-by-tile and normalize: ybnT[ct] is
            # [128 cols, 128 imgs] bf16
            ybnT = []
            for ct in range(16):
                ptr = tpt.tile([128, 128], F16, tag="ptr", bufs=2)
                nc.tensor.transpose(ptr[:], y_sb[:, ct * 128:(ct + 1) * 128],
                                    identb[:])
                yt = tsb.tile([128, 128], BF16, tag=f"ybnT{ct}",
                              name=f"ybnT{ct}")
                nc.vector.tensor_scalar(yt[:], ptr[:], seff[:, ct:ct + 1],
                                        beff[:, ct:ct + 1], ALU.mult,
                                        ALU.add)
                ybnT.append(yt)

            # z = ybn @ Wg1  [128 imgs, 256]
            pz = tps.tile([128, RED], F32, tag="pz", bufs=1)
            for ct in range(16):
                nc.tensor.matmul(pz[:], ybnT[ct][:],
                                 wg1_sb[:, ct * RED:(ct + 1) * RED],
                                 start=(ct == 0), stop=(ct == 15))
            z_sb = tsb.tile([128, RED], F16, tag="z_sb")
            nc.vector.tensor_copy(z_sb[:], pz[:])
            sq2 = tsb.tile([128, RED], F16, tag="sq2")
            nc.vector.tensor_mul(sq2[:], z_sb[:], z_sb[:])
            pstat2 = tps.tile([128, 4], F32, tag="pstat2", bufs=1)
            for j in range(2):
                nc.tensor.matmul(pstat2[:, j:j + 1],
                                 z_sb[:, j * 128:(j + 1) * 128], onesb[:],
                                 start=True, stop=True,
                                 skip_group_check=(j > 0))
                nc.tensor.matmul(pstat2[:, 2 + j:3 + j],
                                 sq2[:, j * 128:(j + 1) * 128], onesb[:],
                                 start=True, stop=True,
                                 skip_group_check=True)
            mu2 = taux.tile([128, 2], F32, tag="mu2")
            nc.vector.tensor_scalar_mul(mu2[:], pstat2[:, 0:2], 1.0 / B)
            ex22 = taux.tile([128, 2], F32, tag="ex22")
            nc.vector.tensor_scalar_mul(ex22[:], pstat2[:, 2:4], 1.0 / B)
            musq2 = taux.tile([128, 2], F32, tag="musq2")
            nc.vector.tensor_mul(musq2[:], mu2[:], mu2[:])
            var2 = taux.tile([128, 2], F32, tag="var2")
            nc.vector.tensor_sub(var2[:], ex22[:], musq2[:])
            sd2 = taux.tile([128, 2], F32, tag="sd2")
            nc.scalar.activation(sd2[:], var2[:], AF.Sqrt, bias=eps1[:, 0:1])
            rstd2 = taux.tile([128, 2], F32, tag="rstd2")
            nc.vector.reciprocal(rstd2[:], sd2[:])
            seff2 = taux.tile([128, 2], F32, tag="seff2")
            nc.vector.tensor_mul(seff2[:], bn2gT_sb[:], rstd2[:])
            mue2 = taux.tile([128, 2], F32, tag="mue2")
            nc.vector.tensor_mul(mue2[:], mu2[:], seff2[:])
            beff2 = taux.tile([128, 2], F32, tag="beff2")
            nc.vector.tensor_sub(beff2[:], bn2bT_sb[:], mue2[:])

            rT = []
            for j in range(2):
                ptr = tpt.tile([128, 128], F16, tag="ptr", bufs=2)
                nc.tensor.transpose(ptr[:], z_sb[:, j * 128:(j + 1) * 128],
                                    identb[:])
                tz = taux.tile([128, 128], F32, tag="tz")
                nc.vector.tensor_scalar(tz[:], ptr[:], seff2[:, j:j + 1],
                                        beff2[:, j:j + 1], ALU.mult,
                                        ALU.add)
                rt_ = tsb.tile([128, 128], BF16, tag=f"rT{j}", name=f"rT{j}")
                nc.vector.tensor_scalar_max(rt_[:], tz[:], 0.0)
                rT.append(rt_)

            # gateT = sigmoid(Wg2^T @ r^T + bg2), oT = ybnT * gateT,
            # outT = W3^T @ oT  -- all in the transposed layout.
            po = tps.tile([NCLS, B], F32, tag="po", bufs=1)
            for m in range(16):
                pg = tpt.tile([128, 128], F32, tag="pg", bufs=2)
                for j in range(2):
                    nc.tensor.matmul(pg[:], wg2_sb[j][:, m * 128:(m + 1) * 128],
                                     rT[j][:], start=(j == 0), stop=(j == 1))
                gt = taux.tile([128, 128], BF16, tag="gt")
                nc.scalar.activation(gt[:], pg[:], AF.Sigmoid,
                                     bias=bg2T_sb[:, m:m + 1])
                ot = taux.tile([128, 128], BF16, tag="ot")
                nc.vector.tensor_mul(ot[:], ybnT[m][:], gt[:])
                nc.tensor.matmul(po[:], w3_sb[:, m * NCLS:(m + 1) * NCLS],
                                 ot[:], start=(m == 0), stop=(m == 15),
                                 skip_group_check=True)
            out_sb = taux.tile([NCLS, B], F32, tag="outp")
            nc.vector.tensor_copy(out_sb[:], po[:])
            nc.sync.dma_start(outT[:], out_sb[:])

    nc.compile()
    return nc


def _host_prep(inputs):
    f32 = np.float32
    bf16 = np.float16
    g = {k: np.asarray(v, dtype=f32) for k, v in inputs.items()}

    x2 = np.transpose(g["x"], (0, 3, 1, 2)).reshape(B, T, POSE)
    Wh = g["W_emb"] @ g["W1"]                       # [144, 4096]
    bh = g["b_emb"] @ g["W1"] + g["b1"]             # [4096]
    C1cat = np.concatenate([g["centroids1"], g["W2"]], axis=1)  # [4096, 520]
    WL = Wh @ C1cat                                 # [144, 520]
    bL = bh @ C1cat
    bL[FS:] += g["b2"]
    wh_aug = np.concatenate([Wh, bh[None, :]], axis=0).astype(bf16)
    wl_aug = np.concatenate([WL, bL[None, :]], axis=0).astype(bf16)
    c2t = np.ascontiguousarray(g["centroids2"][0].T)            # [64, 512]
    c2t2 = np.concatenate([c2t, c2t], axis=0)                   # [128, 512]

    # permute Wcg rows: our flat vlad index is k*FS+f, reference is f*K+k
    new = np.arange(VLAD)
    old = (new % FS) * K + (new // FS)
    wcg_perm = g["Wcg"][old, :].astype(bf16)        # [32768, 2048]

    rmask = np.zeros((128, 1), f32)
    rmask[:T] = 1.0
    rmask[TP:TP + T] = 1.0
    identb_np = np.eye(128, dtype=np.float16)
    identf_np = np.eye(128, dtype=f32)
    onesb_np = np.ones((128, 1), np.float16)
    onesf_np = np.ones((128, 1), f32)
    common = {
        "wh": wh_aug,
        "wl": wl_aug,
        "c2t2": c2t2,
        "wg1": g["Wg1"].astype(bf16),
        "wg2": g["Wg2"].astype(bf16),
        "w3": g["W3"].astype(bf16),
        "bn1gT": np.ascontiguousarray(g["g_bn1"].reshape(16, 128).T),
        "bn1bT": np.ascontiguousarray(g["b_bn1"].reshape(16, 128).T),
        "bn2gT": np.ascontiguousarray(g["g_bn2"].reshape(2, 128).T),
        "bn2bT": np.ascontiguousarray(g["b_bn2"].reshape(2, 128).T),
        "bg2T": np.ascontiguousarray(g["bg2"].reshape(16, 128).T),
        "rowmask": rmask,
        "identb_in": identb_np,
        "identf_in": identf_np,
        "onesb_in": onesb_np,
        "onesf_in": onesf_np,
    }
    in_maps = []
    for c in range(NCORES):
        xs = x2[c * BPC:(c + 1) * BPC]              # [16, 60, 144]
        xp = np.zeros((BPC, TP, POSE), f32)
        xp[:, :T] = xs
        xTf = np.ones((KA, ROWS), f32)
        xTf[:POSE] = xp.reshape(ROWS, POSE).T
        m = dict(common)
        m["xT"] = xTf.astype(bf16)
        m["wcg"] = np.ascontiguousarray(wcg_perm[c * SLC:(c + 1) * SLC, :])
        in_maps.append(m)
    return in_maps, g["b3"]


# vstage row r <-> global batch index (r<64: core r//8 img r%8,
# r>=64: core (r-64)//8 img 8+(r-64)%8)
_PERM = np.zeros(B, np.int64)
for _r in range(B):
    if _r < 64:
        _s, _i = divmod(_r, 8)
        _PERM[_r] = BPC * _s + _i
    else:
        _s, _i = divmod(_r - 64, 8)
        _PERM[_r] = BPC * _s + 8 + _i


def kernel(**inputs):
    if "nc" not in _CACHED:
        _CACHED["nc"] = _build_nc()
    nc = _CACHED["nc"]
    in_maps, b3 = _host_prep(inputs)
    res = bass_utils.run_bass_kernel_spmd(nc, in_maps,
                                          core_ids=list(range(NCORES)))
    _CACHED["last_res"] = res
    out = np.zeros((B, NCLS), np.float32)
    out[_PERM, :] = res.results[0]["outT"].T
    return out + b3[None, :]
